# revision 4
# baseline (speedup 1.0000x reference)
"""MoE gate (DeepSeek-style) on 8 Trainium2 NeuronCores.

Reference semantics (bias == 0, guaranteed by the problem spec):
    logits = x @ w.T                      # [T, 256] fp32
    scores = sigmoid(logits)
    group_scores[g] = sum of top-2 scores in group g (8 groups of 32)
    keep top-4 groups; mask scores outside them to 0
    topk_idx  = top-8 of masked scores (desc, ties -> lowest index)
    topk_weight = scores[topk_idx] / (sum + 1e-20) * 2.5

Sharding: tokens (4*4096 = 16384) split across 8 cores, 2048 each; the
[256, 7168] gate weight is replicated.

Matmul precision: fp32 accuracy at bf16 PE rate via a 3-term hi/lo
split:  x @ w ~= xh@wh + xh@wl + xl@wh  (xh = bf16(x), xl = bf16(x - xh)).
The residual xl@wl term is O(2^-16) relative — below fp32 accumulation
noise (validated: max topk_weight rel err 2.3e-6, zero top-k flips).

Per-core layout: PSUM tile [128 tokens, 256 experts] accumulated over
56 K-blocks; routing runs per 128-token block on ACT (sigmoid) + DVE
(InstMax top-8 / InstMaxIndex / masked ops).
"""

import sys

if "/opt/trn_rl_repo" not in sys.path:
    sys.path.insert(0, "/opt/trn_rl_repo")

import numpy as np
import ml_dtypes

H = 7168
E = 256
TOP_K = 8
N_GROUP = 8
EPG = E // N_GROUP          # 32
TOPK_GROUP = 4
SCALING = 2.5
T_TOTAL = 16384
N_CORES = 8
T_CORE = T_TOTAL // N_CORES  # 2048
HB = H // 128                # 56 k-blocks
SB_TOK = 256                 # tokens per DMA superblock (512B DMA rows)
N_SB = T_CORE // SB_TOK      # 8
TB_PER_SB = SB_TOK // 128    # 2

BF16 = ml_dtypes.bfloat16

_CACHED_NC = None
LAST_RESULTS = None


def _build_nc(repeat=1):
    # `repeat` replicates the whole compute inside one NEFF — used only by
    # the timing harness to measure device time independent of dispatch RTT.
    import concourse.mybir as mybir
    from concourse import bacc
    import concourse.tile as tile

    nc = bacc.Bacc("TRN2", target_bir_lowering=False, debug=False)

    xh_d = nc.dram_tensor("xh", [H, T_CORE], mybir.dt.bfloat16, kind="ExternalInput")
    xl_d = nc.dram_tensor("xl", [H, T_CORE], mybir.dt.bfloat16, kind="ExternalInput")
    wh_d = nc.dram_tensor("wh", [H, E], mybir.dt.bfloat16, kind="ExternalInput")
    wl_d = nc.dram_tensor("wl", [H, E], mybir.dt.bfloat16, kind="ExternalInput")
    oidx_d = nc.dram_tensor("oidx", [T_CORE, TOP_K], mybir.dt.int32, kind="ExternalOutput")
    ow_d = nc.dram_tensor("ow", [T_CORE, TOP_K], mybir.dt.float32, kind="ExternalOutput")

    f32 = mybir.dt.float32
    bf16 = mybir.dt.bfloat16

    with tile.TileContext(nc) as tc:
        with (
            tc.tile_pool(name="wpool", bufs=1) as wpool,
            tc.tile_pool(name="xpool", bufs=2) as xpool,
            tc.tile_pool(name="rpool", bufs=3) as rpool,
            tc.tile_pool(name="stage", bufs=1) as stage,
            tc.tile_pool(name="ppool", bufs=4, space="PSUM") as ppool,
        ):
            # Resident gate weights: [128, 56, 256] per half.
            wh_t = wpool.tile([128, HB, E], bf16, tag="wh")
            wl_t = wpool.tile([128, HB, E], bf16, tag="wl")
            nc.sync.dma_start(out=wh_t[:], in_=wh_d[:].rearrange("(n p) e -> p n e", p=128))
            nc.sync.dma_start(out=wl_t[:], in_=wl_d[:].rearrange("(n p) e -> p n e", p=128))

            # Output staging: one row per partition, one column group per t-block.
            idx_stage = stage.tile([128, T_CORE // 128, TOP_K], mybir.dt.uint32, tag="sidx")
            w_stage = stage.tile([128, T_CORE // 128, TOP_K], f32, tag="sw")

            for rep in range(repeat):
              for s in range(N_SB):
                xh_s = xpool.tile([128, HB, SB_TOK], bf16, tag="xh")
                xl_s = xpool.tile([128, HB, SB_TOK], bf16, tag="xl")
                tsl = slice(s * SB_TOK, (s + 1) * SB_TOK)
                nc.sync.dma_start(
                    out=xh_s[:], in_=xh_d[:, tsl].rearrange("(n p) t -> p n t", p=128)
                )
                nc.sync.dma_start(
                    out=xl_s[:], in_=xl_d[:, tsl].rearrange("(n p) t -> p n t", p=128)
                )

                for tb2 in range(TB_PER_SB):
                    tb = s * TB_PER_SB + tb2      # global t-block id [0, 16)
                    csl = slice(tb2 * 128, (tb2 + 1) * 128)

                    ps = ppool.tile([128, E], f32, tag="ps")
                    for h in range(HB):
                        xh_ap = xh_s[:, h, csl]
                        xl_ap = xl_s[:, h, csl]
                        nc.tensor.matmul(
                            ps[:], xh_ap, wh_t[:, h, :],
                            start=(h == 0), stop=False,
                        )
                        nc.tensor.matmul(ps[:], xh_ap, wl_t[:, h, :], start=False, stop=False)
                        nc.tensor.matmul(
                            ps[:], xl_ap, wh_t[:, h, :],
                            start=False, stop=(h == HB - 1),
                        )

                    # ---- routing for 128 tokens ----
                    sig = rpool.tile([128, E], f32, tag="sig")
                    nc.scalar.activation(sig[:], ps[:], mybir.ActivationFunctionType.Sigmoid)

                    # top-8 per group of 32 -> g8 [128, 8 groups, 8]
                    g8 = rpool.tile([128, N_GROUP, 8], f32, tag="g8")
                    for g in range(N_GROUP):
                        nc.vector.max(
                            out=g8[:, g, :], in_=sig[:, g * EPG:(g + 1) * EPG]
                        )
                    # group score = top1 + top2
                    gs = rpool.tile([128, N_GROUP], f32, tag="gs")
                    nc.vector.tensor_add(gs[:], g8[:, :, 0], g8[:, :, 1])

                    # 4th-largest group score as threshold -> mask
                    gtop = rpool.tile([128, 8], f32, tag="gtop")
                    nc.vector.max(out=gtop[:], in_=gs[:])
                    gmask = rpool.tile([128, N_GROUP], f32, tag="gmask")
                    nc.vector.tensor_scalar(
                        gmask[:], gs[:], gtop[:, TOPK_GROUP - 1:TOPK_GROUP], None,
                        op0=mybir.AluOpType.is_ge,
                    )

                    # masked scores, grouped broadcast multiply
                    tmp = rpool.tile([128, E], f32, tag="tmp")
                    nc.vector.tensor_mul(
                        tmp[:].rearrange("p (g e) -> p g e", g=N_GROUP),
                        sig[:].rearrange("p (g e) -> p g e", g=N_GROUP),
                        gmask[:].unsqueeze(2).to_broadcast([128, N_GROUP, EPG]),
                    )

                    # top-8 experts + indices
                    v8 = rpool.tile([128, TOP_K], f32, tag="v8")
                    i8 = rpool.tile([128, TOP_K], mybir.dt.uint32, tag="i8")
                    nc.vector.max(out=v8[:], in_=tmp[:])
                    nc.vector.max_index(out=i8[:], in_max=v8[:], in_values=tmp[:])

                    # normalize: w8 = v8 / (sum + 1e-20) * 2.5
                    den = rpool.tile([128, 1], f32, tag="den")
                    nc.vector.tensor_reduce(
                        den[:], v8[:], axis=mybir.AxisListType.X, op=mybir.AluOpType.add
                    )
                    nc.vector.tensor_scalar_add(den[:], den[:], 1e-20)
                    rec = rpool.tile([128, 1], f32, tag="rec")
                    nc.vector.reciprocal(rec[:], den[:])
                    nc.vector.tensor_scalar_mul(rec[:], rec[:], SCALING)
                    nc.vector.tensor_scalar_mul(w_stage[:, tb, :], v8[:], rec[:, 0:1])
                    nc.vector.tensor_copy(idx_stage[:, tb, :], i8[:])

            # final result DMAs; token t = tb*128 + p  ->  dst[p, tb, k]
            nc.sync.dma_start(
                out=oidx_d[:].rearrange("(tb p) k -> p tb k", p=128),
                in_=idx_stage[:].bitcast(mybir.dt.int32),
            )
            nc.sync.dma_start(
                out=ow_d[:].rearrange("(tb p) k -> p tb k", p=128),
                in_=w_stage[:],
            )

    nc.compile()
    return nc


def _get_nc():
    global _CACHED_NC
    if _CACHED_NC is None:
        _CACHED_NC = _build_nc()
    return _CACHED_NC


def build_in_maps(hidden_states, weight):
    """Host-side prep: flatten, transpose, bf16 hi/lo split, shard by token."""
    x = np.asarray(hidden_states, dtype=np.float32).reshape(-1, H)
    w = np.asarray(weight, dtype=np.float32)
    assert x.shape == (T_TOTAL, H) and w.shape == (E, H)

    xT = np.ascontiguousarray(x.T)                       # [H, T] fp32
    xh = xT.astype(BF16)                                 # [H, T] bf16
    xl = (xT - xh.astype(np.float32)).astype(BF16)
    wT = np.ascontiguousarray(w.T)                       # [H, E] fp32
    wh = wT.astype(BF16)
    wl = (wT - wh.astype(np.float32)).astype(BF16)

    in_maps = []
    for c in range(N_CORES):
        sl = slice(c * T_CORE, (c + 1) * T_CORE)
        in_maps.append({
            "xh": np.ascontiguousarray(xh[:, sl]),
            "xl": np.ascontiguousarray(xl[:, sl]),
            "wh": wh,
            "wl": wl,
        })
    return in_maps


def kernel(hidden_states, weight, e_score_correction_bias):
    global LAST_RESULTS
    from concourse.bass_utils import run_bass_kernel_spmd

    bias = np.asarray(e_score_correction_bias, dtype=np.float32)
    # The device kernel folds the (spec-guaranteed zero) bias away.
    assert not np.any(bias), "kernel compiled for e_score_correction_bias == 0"

    in_maps = build_in_maps(hidden_states, weight)
    nc = _get_nc()
    res = run_bass_kernel_spmd(nc, in_maps, core_ids=list(range(N_CORES)))
    LAST_RESULTS = res

    topk_idx = np.concatenate([r["oidx"] for r in res.results], axis=0)
    topk_weight = np.concatenate([r["ow"] for r in res.results], axis=0)
    return topk_idx, topk_weight
